# revision 3
# baseline (speedup 1.0000x reference)
"""Multi-head attention (b=4, n=2048, d=768, h=12) on 8 trn2 NeuronCores.

Sharding: (batch x sequence-half) -> 8 shards. Each core gets one batch's
x (rotated by half the sequence for odd cores, which is legal because
attention is permutation-invariant over key positions), computes K/V for
the full sequence and Q for the first 1024 rows, and returns those 1024
output rows. Host concatenates. No collectives needed.

Device algorithm (per core), all fp32:
  1. x^T via PE transposes (x [2048,768] -> xT 6x[128,2048]).
  2. Per head pair: K^T/Q^T/V^T = (x @ W)^T via lhsT=W chunks; V^T is
     PE-transposed back to natural [n, d] layout (+ ones column per head
     for the softmax denominators).
  3. Per head: S^T[k,q] = K^T_slice.T @ Q^T (no P transposes needed),
     P^T = exp(S^T * scale) on ACT, O^T[65,1024] += V_aug.T @ P^T.
     Row 64 of O^T = softmax denominators.
  4. Per (head, qblock): PE-transpose O^T chunk -> O_nat [128q, 65],
     reciprocal of col 64, per-partition scale -> normalized O columns
     accumulated into per-qblock [128, 768] tiles.
  5. Per qblock: PE-transpose back -> proj lhsT chunks, out = O'@Wproj
     (+ bias via broadcast add), DMA out.
"""

import numpy as np

B, N, D, H, HD = 4, 2048, 768, 12, 64
NQ = N // 2            # query rows per core
SCALE = HD ** -0.5
NCORES = 8
NT = N // 128          # 16 key tiles
DT = D // 128          # 6 d-chunks
QT = NQ // 128         # 8 query blocks
NPAIR = H // 2         # 6 head pairs

_RUNNER = None


def _build_program():
    import concourse.bass as bass
    import concourse.tile as tile
    import concourse.mybir as mybir
    from concourse import bacc
    from concourse.masks import make_identity
    from contextlib import ExitStack

    f32 = mybir.dt.float32
    AF = mybir.ActivationFunctionType
    ALU = mybir.AluOpType

    nc = bacc.Bacc("TRN2", target_bir_lowering=False, debug=False,
                   num_devices=NCORES)

    x = nc.dram_tensor("x", [N, D], f32, kind="ExternalInput")
    w_qkv = nc.dram_tensor("w_qkv", [D, 3 * D], f32, kind="ExternalInput")
    w_proj = nc.dram_tensor("w_proj", [D, D], f32, kind="ExternalInput")
    b_proj = nc.dram_tensor("b_proj", [D], f32, kind="ExternalInput")
    y = nc.dram_tensor("y", [NQ, D], f32, kind="ExternalOutput")

    with tile.TileContext(nc) as tc, ExitStack() as ctx:
        singles = ctx.enter_context(tc.tile_pool(name="singles", bufs=1))
        onat_pool = ctx.enter_context(tc.tile_pool(name="onat", bufs=1))
        small_pool = ctx.enter_context(tc.tile_pool(name="small", bufs=4))
        # psum pool for transposes / QKV accumulation / misc (2 banks)
        ps_misc = ctx.enter_context(
            tc.tile_pool(name="ps_misc", bufs=2, space="PSUM"))

        ident = singles.tile([128, 128], f32)
        make_identity(nc, ident)

        bias_bc = singles.tile([128, D], f32)
        b_ap = b_proj.ap()
        b_bcast = bass.AP(tensor=b_ap.tensor, offset=b_ap.offset,
                          ap=[[0, 128]] + list(b_ap.ap))
        nc.gpsimd.dma_start(out=bias_bc[:], in_=b_bcast)

        # per-qblock normalized attention output accumulators [128, 768]
        onat = [onat_pool.tile([128, D], f32, tag=f"onat{qb}", name=f"onat{qb}")
                for qb in range(QT)]

        with tc.tile_pool(name="xt", bufs=1) as xt_pool:
            # ---- Phase A: xT[dt] [128, 2048] = x^T ----
            xT = [xt_pool.tile([128, N], f32, tag=f"xt{dt}", name=f"xt{dt}")
                  for dt in range(DT)]
            with tc.tile_pool(name="xload", bufs=3) as xload:
                for nt in range(NT):
                    xsb = xload.tile([128, D], f32, tag="xsb")
                    nc.sync.dma_start(out=xsb[:],
                                      in_=x[nt * 128:(nt + 1) * 128, :])
                    for dt in range(DT):
                        ps = ps_misc.tile([128, 128], f32, tag="misc")
                        nc.tensor.transpose(
                            ps[:], xsb[:, dt * 128:(dt + 1) * 128], ident[:])
                        nc.vector.tensor_copy(
                            out=xT[dt][:, nt * 128:(nt + 1) * 128], in_=ps[:])

            # ---- Phase C: per head pair: K^T, Q^T, V; then attention ----
            with tc.tile_pool(name="wkq", bufs=14) as wkq_pool, \
                 tc.tile_pool(name="kq", bufs=2) as kq_pool, \
                 tc.tile_pool(name="vsb", bufs=2) as vsb_pool, \
                 tc.tile_pool(name="pt", bufs=2) as pt_pool, \
                 tc.tile_pool(name="otsb", bufs=2) as ot_pool, \
                 tc.tile_pool(name="ps_st", bufs=2, space="PSUM") as ps_st, \
                 tc.tile_pool(name="ps_ot", bufs=1, space="PSUM") as ps_ot:
                for p in range(NPAIR):
                    wk, wq, wv = [], [], []
                    for dt in range(DT):
                        for lst, tag, col0 in ((wk, "wk", D + p * 128),
                                               (wq, "wq", p * 128),
                                               (wv, "wv", 2 * D + p * 128)):
                            t = wkq_pool.tile([128, 128], f32, tag=tag)
                            nc.sync.dma_start(
                                out=t[:],
                                in_=w_qkv[dt * 128:(dt + 1) * 128,
                                          col0:col0 + 128])
                            lst.append(t)

                    # K^T_pair [128, 2048] in halves (2 psum slices at a time)
                    KT = kq_pool.tile([128, N], f32, tag="kt")
                    for half in range(2):
                        pss = [ps_misc.tile([128, 512], f32, tag="misc", name="pss")
                               for _ in range(2)]
                        for dt in range(DT):
                            for i in range(2):
                                ns = half * 2 + i
                                nc.tensor.matmul(
                                    pss[i][:], lhsT=wk[dt][:],
                                    rhs=xT[dt][:, ns * 512:(ns + 1) * 512],
                                    start=(dt == 0), stop=(dt == DT - 1))
                        for i in range(2):
                            ns = half * 2 + i
                            nc.vector.tensor_copy(
                                out=KT[:, ns * 512:(ns + 1) * 512],
                                in_=pss[i][:])

                    # Q^T_pair [128, 1024]
                    QTt = kq_pool.tile([128, NQ], f32, tag="qt")
                    pss = [ps_misc.tile([128, 512], f32, tag="misc", name="pss")
                           for _ in range(2)]
                    for dt in range(DT):
                        for i in range(2):
                            nc.tensor.matmul(
                                pss[i][:], lhsT=wq[dt][:],
                                rhs=xT[dt][:, i * 512:(i + 1) * 512],
                                start=(dt == 0), stop=(dt == DT - 1))
                    for i in range(2):
                        nc.vector.tensor_copy(
                            out=QTt[:, i * 512:(i + 1) * 512], in_=pss[i][:])

                    # V^T_pair [128, 2048], then transpose to natural V_sb
                    # [128, nt, 130]; cols per pair:
                    # [headA 64 | ones | headB 64 | ones]
                    VTt = kq_pool.tile([128, N], f32, tag="vt")
                    for half in range(2):
                        pss = [ps_misc.tile([128, 512], f32, tag="misc", name="pss")
                               for _ in range(2)]
                        for dt in range(DT):
                            for i in range(2):
                                ns = half * 2 + i
                                nc.tensor.matmul(
                                    pss[i][:], lhsT=wv[dt][:],
                                    rhs=xT[dt][:, ns * 512:(ns + 1) * 512],
                                    start=(dt == 0), stop=(dt == DT - 1))
                        for i in range(2):
                            ns = half * 2 + i
                            nc.vector.tensor_copy(
                                out=VTt[:, ns * 512:(ns + 1) * 512],
                                in_=pss[i][:])

                    Vsb = vsb_pool.tile([128, NT, 130], f32, tag="v")
                    nc.vector.memset(Vsb[:, :, 64:65], 1.0)
                    nc.vector.memset(Vsb[:, :, 129:130], 1.0)
                    for kt in range(NT):
                        ps = ps_misc.tile([128, 128], f32, tag="misc")
                        nc.tensor.transpose(
                            ps[:], VTt[:, kt * 128:(kt + 1) * 128], ident[:])
                        dst = Vsb[:, kt, :].rearrange("n (h c) -> n h c", h=2)
                        nc.vector.tensor_copy(
                            out=dst[:, :, 0:64],
                            in_=ps[:].rearrange("n (h c) -> n h c", h=2))

                    for hh in range(2):
                        h = 2 * p + hh
                        base = hh * 64
                        ot = ps_ot.tile([65, NQ], f32, tag="ot")
                        for kt in range(NT):
                            st = ps_st.tile([128, NQ], f32, tag="st")
                            lhsT = KT[base:base + 64, kt * 128:(kt + 1) * 128]
                            for i in range(2):
                                nc.tensor.matmul(
                                    st[:, i * 512:(i + 1) * 512], lhsT=lhsT,
                                    rhs=QTt[base:base + 64,
                                            i * 512:(i + 1) * 512],
                                    start=True, stop=True)
                            pt = pt_pool.tile([128, NQ], f32, tag="pt")
                            nc.scalar.activation(pt[:], st[:], AF.Exp,
                                                 bias=0.0, scale=float(SCALE))
                            vh = Vsb[:, kt, hh * 65: hh * 65 + 65]
                            for i in range(2):
                                nc.tensor.matmul(
                                    ot[:, i * 512:(i + 1) * 512], lhsT=vh,
                                    rhs=pt[:, i * 512:(i + 1) * 512],
                                    start=(kt == 0), stop=(kt == NT - 1))

                        otsb = ot_pool.tile([65, NQ], f32, tag="otsb")
                        nc.vector.tensor_copy(out=otsb[:], in_=ot[:])
                        for qb in range(QT):
                            trp = ps_misc.tile([128, 65], f32, tag="misc")
                            nc.tensor.transpose(
                                trp[:], otsb[:, qb * 128:(qb + 1) * 128],
                                ident[0:65, 0:65])
                            rcp = small_pool.tile([128, 1], f32, tag="rcp")
                            nc.vector.reciprocal(rcp[:], trp[:, 64:65])
                            nc.vector.tensor_scalar_mul(
                                onat[qb][:, h * 64:(h + 1) * 64],
                                trp[:, 0:64], rcp[:])

        # ---- Phase D: output projection ----
        with tc.tile_pool(name="wp", bufs=1) as wp_pool, \
             tc.tile_pool(name="otp", bufs=3) as otp_pool, \
             tc.tile_pool(name="outsb", bufs=3) as out_pool, \
             tc.tile_pool(name="ps_proj", bufs=2, space="PSUM") as ps_proj:
            wp = []
            for dt in range(DT):
                t = wp_pool.tile([128, D], f32, tag=f"wp{dt}")
                nc.sync.dma_start(out=t[:],
                                  in_=w_proj[dt * 128:(dt + 1) * 128, :])
                wp.append(t)

            for qb in range(QT):
                pp = [ps_proj.tile([128, 384], f32, tag="pp", name="pp")
                      for _ in range(2)]
                for dt in range(DT):
                    trp = ps_misc.tile([128, 128], f32, tag="misc")
                    nc.tensor.transpose(
                        trp[:], onat[qb][:, dt * 128:(dt + 1) * 128],
                        ident[:])
                    otp = otp_pool.tile([128, 128], f32, tag="otp")
                    nc.vector.tensor_copy(out=otp[:], in_=trp[:])
                    for i in range(2):
                        nc.tensor.matmul(
                            pp[i][:], lhsT=otp[:],
                            rhs=wp[dt][:, i * 384:(i + 1) * 384],
                            start=(dt == 0), stop=(dt == DT - 1))
                osb = out_pool.tile([128, D], f32, tag="osb")
                for i in range(2):
                    nc.vector.tensor_tensor(
                        osb[:, i * 384:(i + 1) * 384], pp[i][:],
                        bias_bc[:, i * 384:(i + 1) * 384], ALU.add)
                nc.sync.dma_start(out=y[qb * 128:(qb + 1) * 128, :],
                                  in_=osb[:])

    nc.compile()
    return nc


def _make_runner(nc):
    """Cached multi-core PJRT runner (mirrors run_bass_via_pjrt, but the
    jitted callable is built once and reused across kernel() calls)."""
    import jax
    from jax.experimental.shard_map import shard_map
    from jax.sharding import Mesh, PartitionSpec
    import concourse.mybir as mybir
    from concourse.bass2jax import (_bass_exec_p, install_neuronx_cc_hook,
                                    partition_id_tensor)

    install_neuronx_cc_hook()

    partition_name = (nc.partition_id_tensor.name
                      if nc.partition_id_tensor else None)
    in_names, out_names, out_avals, zero_outs = [], [], [], []
    for alloc in nc.m.functions[0].allocations:
        if not isinstance(alloc, mybir.MemoryLocationSet):
            continue
        name = alloc.memorylocations[0].name
        if alloc.kind == "ExternalInput":
            if name != partition_name:
                in_names.append(name)
        elif alloc.kind == "ExternalOutput":
            shape = tuple(alloc.tensor_shape)
            dtype = mybir.dt.np(alloc.dtype)
            out_names.append(name)
            out_avals.append(jax.core.ShapedArray(shape, dtype))
            zero_outs.append(np.zeros(shape, dtype))
    n_params = len(in_names)
    n_outs = len(out_avals)
    all_in_names = list(in_names) + list(out_names)
    if partition_name is not None:
        all_in_names.append(partition_name)

    def _body(*args):
        operands = list(args)
        if partition_name is not None:
            operands.append(partition_id_tensor())
        outs = _bass_exec_p.bind(
            *operands,
            out_avals=tuple(out_avals),
            in_names=tuple(all_in_names),
            out_names=tuple(out_names),
            lowering_input_output_aliases=(),
            sim_require_finite=True,
            sim_require_nnan=True,
            nc=nc,
        )
        return tuple(outs)

    devices = jax.devices()[:NCORES]
    mesh = Mesh(np.asarray(devices), ("core",))
    in_specs = (PartitionSpec("core"),) * (n_params + n_outs)
    out_specs = (PartitionSpec("core"),) * n_outs
    sharded = jax.jit(
        shard_map(_body, mesh=mesh, in_specs=in_specs, out_specs=out_specs,
                  check_rep=False),
        donate_argnums=tuple(range(n_params, n_params + n_outs)),
        keep_unused=True,
    )

    def run(in_maps):
        per_core = [[np.asarray(m[nm]) for nm in in_names] for m in in_maps]
        concat_in = [
            np.concatenate([per_core[c][i] for c in range(NCORES)], axis=0)
            for i in range(n_params)
        ]
        concat_zeros = [
            np.zeros((NCORES * z.shape[0], *z.shape[1:]), z.dtype)
            for z in zero_outs
        ]
        out_arrs = sharded(*concat_in, *concat_zeros)
        return [
            {nm: np.asarray(out_arrs[i]).reshape(NCORES, *out_avals[i].shape)[c]
             for i, nm in enumerate(out_names)}
            for c in range(NCORES)
        ]

    return run


def _get_runner():
    global _RUNNER
    if _RUNNER is None:
        nc = _build_program()
        _RUNNER = _make_runner(nc)
    return _RUNNER


def _make_in_maps(x, w_qkv, w_proj, b_proj):
    x = np.ascontiguousarray(np.asarray(x, dtype=np.float32))
    w_qkv = np.ascontiguousarray(np.asarray(w_qkv, dtype=np.float32))
    w_proj = np.ascontiguousarray(np.asarray(w_proj, dtype=np.float32))
    b_proj = np.ascontiguousarray(np.asarray(b_proj, dtype=np.float32))
    in_maps = []
    for c in range(NCORES):
        b, half = divmod(c, 2)
        xc = x[b] if half == 0 else np.ascontiguousarray(
            np.concatenate([x[b, NQ:], x[b, :NQ]], axis=0))
        in_maps.append({"x": xc, "w_qkv": w_qkv, "w_proj": w_proj,
                        "b_proj": b_proj})
    return in_maps


def kernel(x, w_qkv, w_proj, b_proj):
    run = _get_runner()
    results = run(_make_in_maps(x, w_qkv, w_proj, b_proj))
    out = np.empty((B, N, D), dtype=np.float32)
    for c in range(NCORES):
        b, half = divmod(c, 2)
        out[b, half * NQ:(half + 1) * NQ] = results[c]["y"]
    return out


# revision 6
# speedup vs baseline: 3234.3763x; 3234.3763x over previous
"""Multi-head attention (b=4, n=2048, d=768, h=12) on 8 trn2 NeuronCores.

Sharding: (batch x sequence-half) -> 8 shards. Each core gets one batch's
x (rotated by half the sequence for odd cores, which is legal because
attention is permutation-invariant over key positions), computes K/V for
the full sequence and Q for the first 1024 rows, and returns those 1024
output rows. Host concatenates. No collectives needed.

Device algorithm (per core), all fp32:
  1. x^T via PE transposes (x [2048,768] -> xT 6x[128,2048]).
  2. Per head pair: K^T/Q^T/V^T = (x @ W)^T via lhsT=W chunks; V^T is
     PE-transposed back to natural [n, d] layout (+ ones column per head
     for the softmax denominators).
  3. Per head: S^T[k,q] = K^T_slice.T @ Q^T (no P transposes needed),
     P^T = exp(S^T * scale) on ACT, O^T[65,1024] += V_aug.T @ P^T.
     Row 64 of O^T = softmax denominators.
  4. Per (head, qblock): PE-transpose O^T chunk -> O_nat [128q, 65],
     reciprocal of col 64, per-partition scale -> normalized O columns
     accumulated into per-qblock [128, 768] tiles.
  5. Per qblock: PE-transpose back -> proj lhsT chunks, out = O'@Wproj
     (+ bias via broadcast add), DMA out.
"""

import numpy as np

B, N, D, H, HD = 4, 2048, 768, 12, 64
NQ = N // 2            # query rows per core
SCALE = HD ** -0.5
NCORES = 8
NT = N // 128          # 16 key tiles
DT = D // 128          # 6 d-chunks
QT = NQ // 128         # 8 query blocks
NPAIR = H // 2         # 6 head pairs

_RUNNER = None


def _build_program():
    import concourse.bass as bass
    import concourse.tile as tile
    import concourse.mybir as mybir
    from concourse import bacc
    from concourse.masks import make_identity
    from contextlib import ExitStack

    f32 = mybir.dt.float32
    AF = mybir.ActivationFunctionType
    ALU = mybir.AluOpType
    f32r = mybir.dt.float32r
    r = lambda ap: ap.bitcast(f32r)

    nc = bacc.Bacc("TRN2", target_bir_lowering=False, debug=False,
                   num_devices=NCORES)

    x = nc.dram_tensor("x", [N, D], f32, kind="ExternalInput")
    w_qkv = nc.dram_tensor("w_qkv", [D, 3 * D], f32, kind="ExternalInput")
    w_proj = nc.dram_tensor("w_proj", [D, D], f32, kind="ExternalInput")
    b_proj = nc.dram_tensor("b_proj", [D], f32, kind="ExternalInput")
    y = nc.dram_tensor("y", [NQ, D], f32, kind="ExternalOutput")

    with tile.TileContext(nc) as tc, ExitStack() as ctx:
        singles = ctx.enter_context(tc.tile_pool(name="singles", bufs=1))
        onat_pool = ctx.enter_context(tc.tile_pool(name="onat", bufs=1))
        small_pool = ctx.enter_context(tc.tile_pool(name="small", bufs=4))
        # psum pool for transposes / QKV accumulation / misc (2 banks)
        ps_misc = ctx.enter_context(
            tc.tile_pool(name="ps_misc", bufs=2, space="PSUM"))

        ident = singles.tile([128, 128], f32)
        make_identity(nc, ident)

        bias_bc = singles.tile([128, D], f32)
        b_ap = b_proj.ap()
        b_bcast = bass.AP(tensor=b_ap.tensor, offset=b_ap.offset,
                          ap=[[0, 128]] + list(b_ap.ap))
        nc.gpsimd.dma_start(out=bias_bc[:], in_=b_bcast)

        # per-qblock normalized attention output accumulators [128, 768]
        onat = [onat_pool.tile([128, D], f32, tag=f"onat{qb}", name=f"onat{qb}")
                for qb in range(QT)]

        with tc.tile_pool(name="xt", bufs=1) as xt_pool:
            # ---- Phase A: xT[dt] [128, 2048] = x^T ----
            xT = [xt_pool.tile([128, N], f32r, tag=f"xt{dt}", name=f"xt{dt}")
                  for dt in range(DT)]
            with tc.tile_pool(name="xload", bufs=3) as xload:
                for nt in range(NT):
                    xsb = xload.tile([128, D], f32, tag="xsb")
                    nc.sync.dma_start(out=xsb[:],
                                      in_=x[nt * 128:(nt + 1) * 128, :])
                    for dt in range(DT):
                        ps = ps_misc.tile([128, 128], f32, tag="misc")
                        nc.tensor.transpose(
                            ps[:], xsb[:, dt * 128:(dt + 1) * 128], ident[:])
                        nc.vector.tensor_copy(
                            out=xT[dt][:, nt * 128:(nt + 1) * 128], in_=ps[:])

            # ---- Phase C: per head pair: K^T, Q^T, V; then attention ----
            with tc.tile_pool(name="wkq", bufs=14) as wkq_pool, \
                 tc.tile_pool(name="kq", bufs=2) as kq_pool, \
                 tc.tile_pool(name="vsb", bufs=2) as vsb_pool, \
                 tc.tile_pool(name="pt", bufs=2) as pt_pool, \
                 tc.tile_pool(name="otsb", bufs=2) as ot_pool, \
                 tc.tile_pool(name="ps_st", bufs=2, space="PSUM") as ps_st, \
                 tc.tile_pool(name="ps_ot", bufs=1, space="PSUM") as ps_ot:
                for p in range(NPAIR):
                    wk, wq, wv = [], [], []
                    for dt in range(DT):
                        for lst, tag, col0 in ((wk, "wk", D + p * 128),
                                               (wq, "wq", p * 128),
                                               (wv, "wv", 2 * D + p * 128)):
                            t = wkq_pool.tile([128, 128], f32r, tag=tag)
                            nc.sync.dma_start(
                                out=t[:],
                                in_=w_qkv[dt * 128:(dt + 1) * 128,
                                          col0:col0 + 128].bitcast(f32r))
                            lst.append(t)

                    # K^T_pair [128, 2048] in halves (2 psum slices at a time)
                    KT = kq_pool.tile([128, N], f32r, tag="kt")
                    for half in range(2):
                        pss = [ps_misc.tile([128, 512], f32, tag="misc", name="pss")
                               for _ in range(2)]
                        for dt in range(DT):
                            for i in range(2):
                                ns = half * 2 + i
                                nc.tensor.matmul(
                                    pss[i][:], lhsT=wk[dt][:],
                                    rhs=xT[dt][:, ns * 512:(ns + 1) * 512],
                                    start=(dt == 0), stop=(dt == DT - 1))
                        for i in range(2):
                            ns = half * 2 + i
                            nc.vector.tensor_copy(
                                out=KT[:, ns * 512:(ns + 1) * 512],
                                in_=pss[i][:])

                    # Q^T_pair [128, 1024]
                    QTt = kq_pool.tile([128, NQ], f32r, tag="qt")
                    pss = [ps_misc.tile([128, 512], f32, tag="misc", name="pss")
                           for _ in range(2)]
                    for dt in range(DT):
                        for i in range(2):
                            nc.tensor.matmul(
                                pss[i][:], lhsT=wq[dt][:],
                                rhs=xT[dt][:, i * 512:(i + 1) * 512],
                                start=(dt == 0), stop=(dt == DT - 1))
                    for i in range(2):
                        nc.vector.tensor_copy(
                            out=QTt[:, i * 512:(i + 1) * 512], in_=pss[i][:])

                    # V^T_pair [128, 2048], then transpose to natural V_sb
                    # [128, nt, 130]; cols per pair:
                    # [headA 64 | ones | headB 64 | ones]
                    VTt = kq_pool.tile([128, N], f32, tag="vt")
                    for half in range(2):
                        pss = [ps_misc.tile([128, 512], f32, tag="misc", name="pss")
                               for _ in range(2)]
                        for dt in range(DT):
                            for i in range(2):
                                ns = half * 2 + i
                                nc.tensor.matmul(
                                    pss[i][:], lhsT=wv[dt][:],
                                    rhs=xT[dt][:, ns * 512:(ns + 1) * 512],
                                    start=(dt == 0), stop=(dt == DT - 1))
                        for i in range(2):
                            ns = half * 2 + i
                            nc.vector.tensor_copy(
                                out=VTt[:, ns * 512:(ns + 1) * 512],
                                in_=pss[i][:])

                    Vsb = vsb_pool.tile([128, NT, 130], f32r, tag="v")
                    nc.vector.memset(Vsb[:, :, 64:65].bitcast(f32), 1.0)
                    nc.vector.memset(Vsb[:, :, 129:130].bitcast(f32), 1.0)
                    for kt in range(NT):
                        ps = ps_misc.tile([128, 128], f32, tag="misc")
                        nc.tensor.transpose(
                            ps[:], VTt[:, kt * 128:(kt + 1) * 128], ident[:])
                        dst = Vsb[:, kt, :].rearrange("n (h c) -> n h c", h=2)
                        nc.vector.tensor_copy(
                            out=dst[:, :, 0:64],
                            in_=ps[:].rearrange("n (h c) -> n h c", h=2))

                    for hh in range(2):
                        h = 2 * p + hh
                        base = hh * 64
                        ot = ps_ot.tile([65, NQ], f32, tag="ot")
                        for kt in range(NT):
                            st = ps_st.tile([128, NQ], f32, tag="st")
                            lhsT = KT[base:base + 64, kt * 128:(kt + 1) * 128]
                            for i in range(2):
                                nc.tensor.matmul(
                                    st[:, i * 512:(i + 1) * 512],
                                    lhsT=lhsT,
                                    rhs=QTt[base:base + 64,
                                            i * 512:(i + 1) * 512],
                                    start=True, stop=True)
                            pt = pt_pool.tile([128, NQ], f32r, tag="pt")
                            nc.scalar.activation(pt[:], st[:], AF.Exp,
                                                 bias=0.0, scale=float(SCALE))
                            vh = Vsb[:, kt, hh * 65: hh * 65 + 65]
                            for i in range(2):
                                nc.tensor.matmul(
                                    ot[:, i * 512:(i + 1) * 512], lhsT=vh,
                                    rhs=pt[:, i * 512:(i + 1) * 512],
                                    start=(kt == 0), stop=(kt == NT - 1))

                        otsb = ot_pool.tile([65, NQ], f32, tag="otsb")
                        nc.vector.tensor_copy(out=otsb[:], in_=ot[:])
                        for qb in range(QT):
                            trp = ps_misc.tile([128, 65], f32, tag="misc")
                            nc.tensor.transpose(
                                trp[:], otsb[:, qb * 128:(qb + 1) * 128],
                                ident[0:65, 0:65])
                            rcp = small_pool.tile([128, 1], f32, tag="rcp")
                            nc.vector.reciprocal(rcp[:], trp[:, 64:65])
                            nc.vector.tensor_scalar_mul(
                                onat[qb][:, h * 64:(h + 1) * 64],
                                trp[:, 0:64], rcp[:])

        # ---- Phase D: output projection ----
        with tc.tile_pool(name="wp", bufs=1) as wp_pool, \
             tc.tile_pool(name="otp", bufs=3) as otp_pool, \
             tc.tile_pool(name="outsb", bufs=3) as out_pool, \
             tc.tile_pool(name="ps_proj", bufs=2, space="PSUM") as ps_proj:
            wp = []
            for dt in range(DT):
                t = wp_pool.tile([128, D], f32r, tag=f"wp{dt}")
                nc.sync.dma_start(
                    out=t[:],
                    in_=w_proj[dt * 128:(dt + 1) * 128, :].bitcast(f32r))
                wp.append(t)

            for qb in range(QT):
                pp = [ps_proj.tile([128, 384], f32, tag="pp", name="pp")
                      for _ in range(2)]
                for dt in range(DT):
                    trp = ps_misc.tile([128, 128], f32, tag="misc")
                    nc.tensor.transpose(
                        trp[:], onat[qb][:, dt * 128:(dt + 1) * 128],
                        ident[:])
                    otp = otp_pool.tile([128, 128], f32r, tag="otp")
                    nc.vector.tensor_copy(out=otp[:], in_=trp[:])
                    for i in range(2):
                        nc.tensor.matmul(
                            pp[i][:], lhsT=otp[:],
                            rhs=wp[dt][:, i * 384:(i + 1) * 384],
                            start=(dt == 0), stop=(dt == DT - 1))
                osb = out_pool.tile([128, D], f32, tag="osb")
                for i in range(2):
                    nc.vector.tensor_tensor(
                        osb[:, i * 384:(i + 1) * 384], pp[i][:],
                        bias_bc[:, i * 384:(i + 1) * 384], ALU.add)
                nc.sync.dma_start(out=y[qb * 128:(qb + 1) * 128, :],
                                  in_=osb[:])

    nc.compile()
    return nc


def _make_runner(nc):
    """Cached multi-core PJRT runner (mirrors run_bass_via_pjrt, but the
    jitted callable is built once and reused across kernel() calls)."""
    import jax
    from jax.experimental.shard_map import shard_map
    from jax.sharding import Mesh, PartitionSpec
    import concourse.mybir as mybir
    from concourse.bass2jax import (_bass_exec_p, install_neuronx_cc_hook,
                                    partition_id_tensor)

    install_neuronx_cc_hook()

    partition_name = (nc.partition_id_tensor.name
                      if nc.partition_id_tensor else None)
    in_names, out_names, out_avals, zero_outs = [], [], [], []
    for alloc in nc.m.functions[0].allocations:
        if not isinstance(alloc, mybir.MemoryLocationSet):
            continue
        name = alloc.memorylocations[0].name
        if alloc.kind == "ExternalInput":
            if name != partition_name:
                in_names.append(name)
        elif alloc.kind == "ExternalOutput":
            shape = tuple(alloc.tensor_shape)
            dtype = mybir.dt.np(alloc.dtype)
            out_names.append(name)
            out_avals.append(jax.core.ShapedArray(shape, dtype))
            zero_outs.append(np.zeros(shape, dtype))
    n_params = len(in_names)
    n_outs = len(out_avals)
    all_in_names = list(in_names) + list(out_names)
    if partition_name is not None:
        all_in_names.append(partition_name)

    def _body(*args):
        operands = list(args)
        if partition_name is not None:
            operands.append(partition_id_tensor())
        outs = _bass_exec_p.bind(
            *operands,
            out_avals=tuple(out_avals),
            in_names=tuple(all_in_names),
            out_names=tuple(out_names),
            lowering_input_output_aliases=(),
            sim_require_finite=True,
            sim_require_nnan=True,
            nc=nc,
        )
        return tuple(outs)

    devices = jax.devices()[:NCORES]
    mesh = Mesh(np.asarray(devices), ("core",))
    in_specs = (PartitionSpec("core"),) * (n_params + n_outs)
    out_specs = (PartitionSpec("core"),) * n_outs
    sharded = jax.jit(
        shard_map(_body, mesh=mesh, in_specs=in_specs, out_specs=out_specs,
                  check_rep=False),
        donate_argnums=tuple(range(n_params, n_params + n_outs)),
        keep_unused=True,
    )

    def run(in_maps):
        per_core = [[np.asarray(m[nm]) for nm in in_names] for m in in_maps]
        concat_in = [
            np.concatenate([per_core[c][i] for c in range(NCORES)], axis=0)
            for i in range(n_params)
        ]
        concat_zeros = [
            np.zeros((NCORES * z.shape[0], *z.shape[1:]), z.dtype)
            for z in zero_outs
        ]
        out_arrs = sharded(*concat_in, *concat_zeros)
        return [
            {nm: np.asarray(out_arrs[i]).reshape(NCORES, *out_avals[i].shape)[c]
             for i, nm in enumerate(out_names)}
            for c in range(NCORES)
        ]

    return run


def _get_runner():
    global _RUNNER
    if _RUNNER is None:
        nc = _build_program()
        _RUNNER = _make_runner(nc)
    return _RUNNER


def _make_in_maps(x, w_qkv, w_proj, b_proj):
    x = np.ascontiguousarray(np.asarray(x, dtype=np.float32))
    w_qkv = np.ascontiguousarray(np.asarray(w_qkv, dtype=np.float32))
    w_proj = np.ascontiguousarray(np.asarray(w_proj, dtype=np.float32))
    b_proj = np.ascontiguousarray(np.asarray(b_proj, dtype=np.float32))
    in_maps = []
    for c in range(NCORES):
        b, half = divmod(c, 2)
        xc = x[b] if half == 0 else np.ascontiguousarray(
            np.concatenate([x[b, NQ:], x[b, :NQ]], axis=0))
        in_maps.append({"x": xc, "w_qkv": w_qkv, "w_proj": w_proj,
                        "b_proj": b_proj})
    return in_maps


def kernel(x, w_qkv, w_proj, b_proj):
    run = _get_runner()
    results = run(_make_in_maps(x, w_qkv, w_proj, b_proj))
    out = np.empty((B, N, D), dtype=np.float32)
    for c in range(NCORES):
        b, half = divmod(c, 2)
        out[b, half * NQ:(half + 1) * NQ] = results[c]["y"]
    return out
